# revision 28
# baseline (speedup 1.0000x reference)
"""RGCN basis-decomposed message passing on 8 TRN2 NeuronCores (v3).

Math: out = A @ y,  y = x_flat @ w_perm  (linearity collapse: the edge
aggregation is relation-independent once y is precomputed).

v3 design (vs v2 baseline at 524us):
  - y is stored as bf16 NODE-PAIRS: table row q holds [y[node p] | y[node
    p+64]] of one 128-node group -> 256B gather elements, 25088 rows total
    (fits a single int16 index window). AllGather traffic halved vs f32.
  - The y AllGather runs in 5 group-chunks pipelined behind phase A, so
    the first edge gathers start at ~20us instead of ~100us.
  - Edge tiles are (dst-group, sigma)-pure where sigma = src partition
    // 64 selects the pair half: the half-select is a free AP slice of
    the gathered buffer. Edge weights are applied by one batched DVE
    multiply per run; the scatter one-hot by one batched DVE is_equal.
    The Scalar engine only copies finished PSUM accumulators out.
  - All 49 dst-group accumulators are PSUM-resident simultaneously
    (7 banks x 8 slices + 1 bank for phase A), so matmuls accumulate
    per-group across AllGather epochs with no barrier.
  - Gathers are merged into ~4 large dma_gather calls per epoch over the
    4 SWDGE queues. The measured SWDGE wall (~2.9ns/descriptor agg) is
    the remaining floor.
"""

import math
import sys
from contextlib import ExitStack

for _p in ("/opt/trn_rl_repo",):
    if _p not in sys.path:
        sys.path.insert(0, _p)

import ml_dtypes
import numpy as np

import concourse.bacc as bacc
import concourse.bass as bass
import concourse.mybir as mybir
from concourse import library_config
from concourse.bass_utils import run_bass_kernel_spmd

F32 = mybir.dt.float32
BF16 = mybir.dt.bfloat16
I16 = mybir.dt.int16
NPBF16 = ml_dtypes.bfloat16
P = 128
COPY = mybir.ActivationFunctionType.Copy


class Cfg:
    def __init__(self, N, E, D=64, R=8, C=8, nchunk=5, ring=96):
        self.N, self.E, self.D, self.R, self.C = N, E, D, R, C
        assert N % C == 0
        self.NS = N // C                      # nodes per core
        self.G = math.ceil(self.NS / P)       # 128-node dst groups per core
        self.NS_PAD = self.G * P
        self.K = D * R // P                   # contraction chunks in phase A
        assert D * R % P == 0
        # AllGather chunks over groups
        base = self.G // nchunk
        sizes = [base + (1 if i < self.G % nchunk else 0) for i in range(nchunk)]
        self.chunks = []
        a = 0
        for s in sizes:
            self.chunks.append((a, a + s))
            a += s
        self.NCHUNK = nchunk
        # pair-table rows: chunk-major; chunk i holds rows for groups [a,b)
        # row(c, p2, g) = base_i + (c*64 + p2)*w_i + (g - a_i)
        self.tbase = np.zeros(nchunk + 1, np.int64)
        for i, (a, b) in enumerate(self.chunks):
            self.tbase[i + 1] = self.tbase[i] + C * 64 * (b - a)
        self.NTAB = int(self.tbase[-1])       # 25088 pair rows
        assert self.NTAB < 32768
        self.RING = ring


def plan_and_pack(cfg, x, edge_src, edge_dst, edge_weight, w_bases, w_rel):
    """Host preprocessing: sharding, tiling, schedule. Returns (plan, in_maps)."""
    C, NS, G, D, K = cfg.C, cfg.NS, cfg.G, cfg.D, cfg.K
    NCHUNK = cfg.NCHUNK

    es = edge_src.astype(np.int64)
    ed = edge_dst.astype(np.int64)
    ew = edge_weight.astype(np.float32)

    # dense weights: w[r] = sum_b w_rel[r,b] w_bases[b];  w_perm[(i,r), o]
    w = np.einsum("rb,bio->rio", w_rel.astype(np.float64),
                  w_bases.astype(np.float64)).astype(np.float32)
    w_perm = np.ascontiguousarray(w.transpose(1, 0, 2).reshape(D * cfg.R, D))
    wp = np.ascontiguousarray(
        w_perm.reshape(K, P, D).transpose(1, 0, 2).reshape(P, K * D)
    ).astype(NPBF16)

    # src -> (pair-table row, sigma, chunk)
    src_c, src_l = es // NS, es % NS
    g_s, p_s = src_l // P, src_l % P
    p2, sigma = p_s % 64, p_s // 64
    chunk_of = np.zeros(G, np.int64)
    for i, (a, b) in enumerate(cfg.chunks):
        chunk_of[a:b] = i
    ci = chunk_of[g_s]
    a_i = np.array([a for (a, b) in cfg.chunks])[ci]
    w_i = np.array([b - a for (a, b) in cfg.chunks])[ci]
    trow = cfg.tbase[ci] + (src_c * 64 + p2) * w_i + (g_s - a_i)

    dst_c, dst_l = ed // NS, ed % NS
    g_d, dloc = dst_l // P, dst_l % P

    # per-core edge ordering: (dst group, sigma), then src chunk
    key = (g_d * 2 + sigma)
    per_core = []
    cnt = np.zeros((C, G * 2), np.int64)
    for c in range(C):
        m = np.where(dst_c == c)[0]
        order = m[np.lexsort((ci[m], key[m]))]
        per_core.append(order)
        b = np.bincount(key[m], minlength=G * 2)
        cnt[c] = b
    bounds = np.zeros((C, G * 2 + 1), np.int64)
    bounds[:, 1:] = np.cumsum(cnt, axis=1)

    # SPMD-uniform tile counts per (g, sigma)
    Tgs = np.maximum(0, np.ceil(cnt.max(axis=0) / P).astype(np.int64))  # [G*2]
    # tile ready-epoch = max over cores of chunk of last edge in tile
    tiles = []   # (g, sigma, t, epoch)
    for gs in range(G * 2):
        g, sg = gs // 2, gs % 2
        for t in range(int(Tgs[gs])):
            ep = 0
            for c in range(C):
                n = cnt[c, gs]
                if n <= t * P:
                    continue
                last = min(n, (t + 1) * P) - 1
                e_idx = per_core[c][bounds[c, gs] + last]
                ep = max(ep, int(ci[e_idx]))
            tiles.append((g, sg, t, ep))
    # global order: by (epoch, g, sigma, t)
    tiles.sort(key=lambda z: (z[3], z[0], z[1], z[2]))
    TT = len(tiles)
    pos_of = {}
    for pos, (g, sg, t, ep) in enumerate(tiles):
        pos_of[(g, sg, t)] = pos

    # runs: consecutive tiles with same (g, sigma) (epoch-sorted so same ep)
    runs = []   # dict(g, sigma, pos, nt, epoch)
    for pos, (g, sg, t, ep) in enumerate(tiles):
        if runs and runs[-1]["g"] == g and runs[-1]["sigma"] == sg \
                and runs[-1]["pos"] + runs[-1]["nt"] == pos \
                and runs[-1]["epoch"] == ep:
            runs[-1]["nt"] += 1
        else:
            runs.append(dict(g=g, sigma=sg, pos=pos, nt=1, epoch=ep))

    # explicit ring positions: runs never straddle the ring boundary
    RING = cfg.RING
    rpos = 0
    for r in runs:
        if rpos + r["nt"] > RING:
            rpos = 0
        r["posr"] = rpos
        rpos += r["nt"]

    # calls: pack runs per epoch into ~4 calls; calls are ring-contiguous
    calls = []  # dict(epoch, pos, nt, posr, queue, qidx, guard)
    qn = 0
    qidx = [0, 0, 0, 0]
    for ep in range(NCHUNK):
        ep_runs = [r for r in runs if r["epoch"] == ep]
        if not ep_runs:
            continue
        tot = sum(r["nt"] for r in ep_runs)
        tgt = max(1, math.ceil(tot / 4))
        cur = None
        for r in ep_runs:
            contig = cur is not None and \
                r["posr"] == cur["posr"] + cur["nt"]
            if cur is None or cur["nt"] >= tgt or not contig:
                if cur is not None:
                    calls.append(cur)
                cur = dict(epoch=ep, pos=r["pos"], nt=0, posr=r["posr"],
                           queue=qn % 4, qidx=qidx[qn % 4])
                qidx[qn % 4] += 1
                qn += 1
            cur["nt"] += r["nt"]
            r["call"] = cur
        calls.append(cur)
    # indices: s_mm counts RUNS (inc 1 each), s_dve counts DVE OPS
    for ridx, r in enumerate(runs):
        r["ridx"] = ridx
    for kidx, ccall in enumerate(calls):
        ccall["kidx"] = kidx
    for r in runs:
        r["dve_target"] = 2 * (r["ridx"] + 1)
    # ring-reuse guards in run-count units
    slot_need = np.zeros(RING, np.int64)
    for ccall in calls:
        a, b = ccall["posr"], ccall["posr"] + ccall["nt"]
        ccall["guard"] = int(slot_need[a:b].max())
    # two passes: guards above read pre-update state per call in order
    slot_need[:] = 0
    for ccall in calls:
        a, b = ccall["posr"], ccall["posr"] + ccall["nt"]
        ccall["guard"] = int(slot_need[a:b].max())
        for r in runs:
            if r.get("call") is ccall:
                slot_need[r["posr"]: r["posr"] + r["nt"]] = r["ridx"] + 1

    # group first/last tiles + completion order
    first_pos = np.full(G, TT, np.int64)
    last_pos = np.full(G, -1, np.int64)
    for pos, (g, sg, t, ep) in enumerate(tiles):
        first_pos[g] = min(first_pos[g], pos)
        last_pos[g] = max(last_pos[g], pos)
    comp_order = sorted(range(G), key=lambda g: last_pos[g])
    slot_of = np.zeros(G, np.int64)
    for j, g in enumerate(comp_order):
        slot_of[g] = j
    # s_mm is incremented by nt at end of each run; cumulative tiles through
    # the run containing pos:


    run_end = np.zeros(TT, np.int64)   # for tile pos: runs done incl its run
    for r in runs:
        run_end[r["pos"]: r["pos"] + r["nt"]] = r["ridx"] + 1

    # per-core data arrays
    in_maps = []
    for c in range(C):
        gidx = np.zeros((P, 8 * TT), np.int16)
        dst_a = np.full((P, TT), 999.0, np.float32)
        wgt_a = np.zeros((P, TT), np.float32)
        ord_c = per_core[c]
        for gs in range(G * 2):
            g, sg = gs // 2, gs % 2
            lo, hi = bounds[c, gs], bounds[c, gs + 1]
            n = hi - lo
            for t in range(int(Tgs[gs])):
                pos = pos_of[(g, sg, t)]
                s0, s1 = t * P, min(n, (t + 1) * P)
                L = max(0, s1 - s0)
                vals = np.zeros(P, np.int64)
                if L > 0:
                    eidx = ord_c[lo + s0: lo + s1]
                    vals[:L] = trow[eidx]
                    dst_a[:L, pos] = dloc[eidx]
                    wgt_a[:L, pos] = ew[eidx]
                gidx[:, 8 * pos: 8 * (pos + 1)] = np.tile(
                    vals.astype(np.int16).reshape(8, 16).T, (8, 1))
        xs = x[c * NS:(c + 1) * NS].reshape(NS, D * cfg.R)
        xT = np.zeros((D * cfg.R, cfg.NS_PAD), NPBF16)
        xT[:, :NS] = xs.T.astype(NPBF16)
        iota = np.broadcast_to(np.arange(P, dtype=np.float32),
                               (P, P)).astype(NPBF16).copy()
        in_maps.append({
            "xT": xT, "wp": wp.copy(), "gidx": gidx,
            "dstloc": dst_a.astype(NPBF16), "wgt": wgt_a.astype(NPBF16),
            "iota": iota,
        })

    plan = {"tiles": tiles, "runs": runs, "calls": calls, "TT": TT,
            "first_pos": first_pos.tolist(), "last_pos": last_pos.tolist(),
            "comp_order": comp_order, "slot_of": slot_of.tolist(),
            "run_end": run_end.tolist()}
    return plan, in_maps


def build_nc(cfg, plan):
    C, G, D, K, NCHUNK = cfg.C, cfg.G, cfg.D, cfg.K, cfg.NCHUNK
    NS_PAD, TT, RING = cfg.NS_PAD, plan["TT"], cfg.RING
    runs, calls, tiles = plan["runs"], plan["calls"], plan["tiles"]
    first_pos, last_pos = plan["first_pos"], plan["last_pos"]
    comp_order, run_end = plan["comp_order"], plan["run_end"]
    chunks, tbase = cfg.chunks, cfg.tbase

    nc = bacc.Bacc("TRN2", num_swdge_queues=4,
               detect_race_conditions=False)

    xT_d = nc.declare_dram_parameter("xT", [K * P, NS_PAD], BF16, isOutput=False)
    wp_d = nc.declare_dram_parameter("wp", [P, K * D], BF16, isOutput=False)
    gidx_d = nc.declare_dram_parameter("gidx", [P, 8 * TT], I16, isOutput=False)
    dst_d = nc.declare_dram_parameter("dstloc", [P, TT], BF16, isOutput=False)
    wgt_d = nc.declare_dram_parameter("wgt", [P, TT], BF16, isOutput=False)
    iota_d = nc.declare_dram_parameter("iota", [P, P], BF16, isOutput=False)
    out_d = nc.declare_dram_parameter("out", [P, G * D], F32, isOutput=True)

    # pair table: per chunk i (groups [a,b), w=b-a): [64, w, 2, 64] bf16
    y_own = nc.dram_tensor("y_own", [64 * G * 2 * D], BF16)
    y_all = nc.dram_tensor("y_all", [C * 64 * G * 2 * D], BF16,
                           addr_space="Shared")
    obase = [int(tbase[i]) // C * 2 * D for i in range(NCHUNK + 1)]
    abase = [int(tbase[i]) * 2 * D for i in range(NCHUNK + 1)]

    with ExitStack() as top:
        sem = top.enter_context
        s_wp = sem(nc.semaphore("s_wp"))
        s_xt = [sem(nc.semaphore(f"s_xt{i}")) for i in range(NCHUNK)]
        s_meta = sem(nc.semaphore("s_meta"))
        s_mmA = sem(nc.semaphore("s_mmA"))
        s_yA = sem(nc.semaphore("s_yA"))
        s_ydma = [sem(nc.semaphore(f"s_ydma{i}")) for i in range(NCHUNK)]
        s_cc = sem(nc.semaphore("s_cc"))
        s_g = [sem(nc.semaphore(f"s_g{q}")) for q in range(4)]
        s_dve = sem(nc.semaphore("s_dve"))
        s_mm = sem(nc.semaphore("s_mm"))
        s_po = sem(nc.semaphore("s_po"))
        s_od = sem(nc.semaphore("s_od"))
        s_ms = sem(nc.semaphore("s_ms"))
        s_pz = sem(nc.semaphore("s_pz"))

        sb = top.enter_context
        gidx_sb = sb(nc.sbuf_tensor("gidx_sb", [P, 8 * TT], I16))
        dst_sb = sb(nc.sbuf_tensor("dst_sb", [P, TT], BF16))
        wgt_sb = sb(nc.sbuf_tensor("wgt_sb", [P, TT], BF16))
        iota_sb = sb(nc.sbuf_tensor("iota_sb", [P, P], BF16))
        xT_sb = sb(nc.sbuf_tensor("xT_sb", [P, K, NS_PAD], BF16))
        wp_sb = sb(nc.sbuf_tensor("wp_sb", [P, K * D], BF16))
        y_sb = sb(nc.sbuf_tensor("y_sb", [P, G, D], BF16))
        out_sb = sb(nc.sbuf_tensor("out_sb", [P, G, D], F32))
        gbuf = sb(nc.sbuf_tensor("gbuf", [P, RING, 2 * D], BF16))
        mbuf = sb(nc.sbuf_tensor("mbuf", [P, RING, P], BF16))
        msg = sb(nc.sbuf_tensor("msg", [P, RING, D], BF16))
        psA = sb(nc.psum_tensor("psA", [P, 4, D], F32))
        psC = sb(nc.psum_tensor("psC", [P, G, D], F32))

        def psc(g):
            return psC[:, g, :]

        batches = []      # (gstart, gend) — phase A groups, <=4, chunk-pure
        for (a, b) in chunks:
            g0 = a
            while g0 < b:
                batches.append((g0, min(b, g0 + 4)))
                g0 += 4
        nb_thru = []      # cumulative batch count through each chunk
        nb = 0
        for (a, b) in chunks:
            nb += math.ceil((b - a) / 4)
            nb_thru.append(nb)
        y_rows = y_all.rearrange("(q d) -> q d", d=2 * D)
        blockC = top.enter_context(nc.Block())

        @blockC.sync
        def _(sync):
            sync.dma_start(out=wp_sb[:], in_=wp_d[:]).then_inc(s_wp, 16)
            xr = xT_d.rearrange("(k p) n -> p k n", p=P)
            for i, (a, b) in enumerate(chunks):
                sync.dma_start(
                    out=xT_sb[:, :, a * P:b * P], in_=xr[:, :, a * P:b * P],
                ).then_inc(s_xt[i], 16)
            sync.dma_start(out=gidx_sb[:], in_=gidx_d[:]).then_inc(s_meta, 16)
            sync.dma_start(out=dst_sb[:], in_=dst_d[:]).then_inc(s_meta, 16)
            sync.dma_start(out=wgt_sb[:], in_=wgt_d[:]).then_inc(s_meta, 16)
            sync.dma_start(out=iota_sb[:], in_=iota_d[:]).then_inc(s_meta, 16)
            # y pair-table writes per chunk: two partition-halves
            for i, (a, b) in enumerate(chunks):
                w = b - a
                reg = y_own[obase[i]:obase[i + 1]].rearrange(
                    "(p w s d) -> p w s d", p=64, w=w, s=2, d=D)
                sync.wait_ge(s_yA, nb_thru[i])
                sync.dma_start(
                    out=reg[:, :, 0, :], in_=y_sb[0:64, a:b, :],
                ).then_inc(s_ydma[i], 16)
                sync.dma_start(
                    out=reg[:, :, 1, :], in_=y_sb[64:128, a:b, :],
                ).then_inc(s_ydma[i], 16)
            # output DMAs in completion-slot batches
            nod = 0
            step = math.ceil(G / NCHUNK)
            done = 0
            for a in range(0, G, step):
                b = min(G, a + step)
                # wait until the b-th completed group's copy is done
                sync.wait_ge(s_po, b)
                sync.dma_start(
                    out=out_d[:, a * D:b * D], in_=out_sb[:, a:b, :],
                ).then_inc(s_od, 16)
                nod += 16
            sync.wait_ge(s_od, nod)

        @blockC.tensor
        def _(tensor):
            tensor.wait_ge(s_wp, 16)
            ci = 0
            for bi, (ga, gb) in enumerate(batches):
                while ga >= chunks[ci][1]:
                    ci += 1
                if ga == chunks[ci][0]:
                    tensor.wait_ge(s_xt[ci], 16)
                if bi >= 1:
                    tensor.wait_ge(s_yA, bi)
                for j, nt in enumerate(range(ga, gb)):
                    for k in range(K):
                        mm = tensor.matmul(
                            psA[:, j, :],
                            xT_sb[:, k, nt * P:(nt + 1) * P],
                            wp_sb[:, k * D:(k + 1) * D],
                            start=(j == 0 and k == 0),
                            stop=(nt == gb - 1 and k == K - 1),
                        )
                mm.then_inc(s_mmA, 1)
            # phase C
            tensor.wait_ge(s_pz, 1)
            for r in runs:
                g, pos, nt = r["g"], r["pos"], r["nt"]
                tensor.wait_ge(s_dve, r["dve_target"])
                for i in range(nt):
                    posr = r["posr"] + i
                    mm = tensor.matmul(
                        psc(g),
                        mbuf[:, posr, :],
                        msg[:, posr, :],
                        start=False, stop=False, skip_group_check=True,
                    )
                mm.then_inc(s_mm, 1)

        @blockC.scalar
        def _(scalar):
            # phase A: per-batch psum -> bf16 y_sb copy
            for bi, (ga, gb) in enumerate(batches):
                scalar.wait_ge(s_mmA, bi + 1)
                scalar.activation(
                    out=y_sb[:, ga:gb, :], in_=psA[:, 0:gb - ga, :],
                    func=COPY,
                ).then_inc(s_yA, 1)
            # phase C: completed-group copies, in completion order
            for j, g in enumerate(comp_order):
                scalar.wait_ge(s_mm, int(run_end[last_pos[g]]))
                scalar.copy(out_sb[:, j, :], psc(g)).then_inc(s_po, 1)

        @blockC.vector
        def _(vector):
            vector.memset(psC[:], 0.0).then_inc(s_pz, 1)
            vector.memset(gbuf[:], 0.0).then_inc(s_ms, 1)
            vector.wait_ge(s_meta, 64)
            cur_call = None
            for r in runs:
                pos, nt, posr, sg2 = r["pos"], r["nt"], r["posr"], r["sigma"]
                cc = r["call"]
                if cc is not cur_call:
                    vector.wait_ge(s_g[cc["queue"]], 16 * (cc["qidx"] + 1))
                    cur_call = cc
                dstb = dst_sb[:, pos:pos + nt].unsqueeze(2).to_broadcast(
                    [P, nt, P])
                iotb = iota_sb[:, :].unsqueeze(1).to_broadcast([P, nt, P])
                vector.tensor_tensor(
                    out=mbuf[:, posr:posr + nt, :], in0=dstb, in1=iotb,
                    op=mybir.AluOpType.is_equal,
                ).then_inc(s_dve, 1)
                wgtb = wgt_sb[:, pos:pos + nt].unsqueeze(2).to_broadcast(
                    [P, nt, D])
                vector.tensor_tensor(
                    out=msg[:, posr:posr + nt, :],
                    in0=gbuf[:, posr:posr + nt, sg2 * D:(sg2 + 1) * D],
                    in1=wgtb,
                    op=mybir.AluOpType.mult,
                ).then_inc(s_dve, 1)

        @blockC.gpsimd
        def _(gpsimd):
            gpsimd.load_library(library_config.mlp)

            def ag(i):
                gpsimd.wait_ge(s_ydma[i], 32)
                gpsimd.collective_compute(
                    "AllGather",
                    mybir.AluOpType.bypass,
                    replica_groups=[list(range(C))],
                    ins=[y_own[obase[i]:obase[i + 1]].opt()],
                    outs=[y_all[abase[i]:abase[i + 1]].opt()],
                ).then_inc(s_cc)

            def do_call(ccall):
                pos, nt = ccall["pos"], ccall["nt"]
                gpsimd.wait_ge(s_cc, ccall["epoch"] + 1)
                if ccall["guard"] > 0:
                    gpsimd.wait_ge(s_mm, ccall["guard"])
                posr = ccall["posr"]
                gpsimd.dma_gather(
                    gbuf[:, posr:posr + nt, :],
                    y_rows[0:int(tbase[ccall["epoch"] + 1]), :],
                    gidx_sb[:, 8 * pos: 8 * (pos + nt)],
                    nt * P, nt * P, 2 * D,
                    single_packet=False, queue_num=ccall["queue"],
                ).then_inc(s_g[ccall["queue"]], 16)

            by_ep = [[cl for cl in calls if cl["epoch"] == ep]
                     for ep in range(NCHUNK)]
            gpsimd.wait_ge(s_ms, 1)
            ag(0)
            if NCHUNK > 1:
                ag(1)
            for ep in range(NCHUNK):
                for ccall in by_ep[ep]:
                    do_call(ccall)
                if ep + 2 < NCHUNK:
                    ag(ep + 2)

    nc.compile()
    return nc


def _assemble(cfg, plan, outs):
    D, G, NS = cfg.D, cfg.G, cfg.NS
    comp_order = plan["comp_order"]
    full = np.empty((cfg.N, D), np.float32)
    for c in range(cfg.C):
        o = outs[c]["out"].reshape(P, G, D)
        per_g = np.empty((G, P, D), np.float32)
        for j, g in enumerate(comp_order):
            per_g[g] = o[:, j, :]
        flat = per_g.transpose(0, 1, 2).reshape(cfg.NS_PAD, D)
        full[c * NS:(c + 1) * NS] = flat[:NS]
    return full


def gnn_kernel(x, edge_src, edge_dst, edge_weight, w_bases, w_rel,
               cfg=None, trace=False):
    if cfg is None:
        cfg = Cfg(N=50000, E=800000)
    plan, in_maps = plan_and_pack(cfg, np.asarray(x), np.asarray(edge_src),
                                  np.asarray(edge_dst), np.asarray(edge_weight),
                                  np.asarray(w_bases), np.asarray(w_rel))
    nc = build_nc(cfg, plan)
    res = run_bass_kernel_spmd(nc, in_maps, list(range(cfg.C)), trace=trace)
    return _assemble(cfg, plan, res.results), res


def kernel(x, edge_src, edge_dst, edge_weight, w_bases, w_rel):
    """Full inputs in, full output out. Shards across 8 NeuronCores inside."""
    full, _ = gnn_kernel(x, edge_src, edge_dst, edge_weight, w_bases, w_rel)
    return full


# revision 29
# speedup vs baseline: 1.2716x; 1.2716x over previous
"""RGCN basis-decomposed message passing on 8 TRN2 NeuronCores (v3).

Math: out = A @ y,  y = x_flat @ w_perm  (linearity collapse: the edge
aggregation is relation-independent once y is precomputed).

v3 design (vs v2 baseline at 524us):
  - y is stored as bf16 NODE-PAIRS: table row q holds [y[node p] | y[node
    p+64]] of one 128-node group -> 256B gather elements, 25088 rows total
    (fits a single int16 index window). AllGather traffic halved vs f32.
  - The y AllGather runs in 5 group-chunks pipelined behind phase A, so
    the first edge gathers start at ~20us instead of ~100us.
  - Edge tiles are (dst-group, sigma)-pure where sigma = src partition
    // 64 selects the pair half: the half-select is a free AP slice of
    the gathered buffer. Edge weights are applied by one batched DVE
    multiply per run; the scatter one-hot by one batched DVE is_equal.
    The Scalar engine only copies finished PSUM accumulators out.
  - All 49 dst-group accumulators are PSUM-resident simultaneously
    (7 banks x 8 slices + 1 bank for phase A), so matmuls accumulate
    per-group across AllGather epochs with no barrier.
  - Gathers are merged into ~4 large dma_gather calls per epoch over the
    4 SWDGE queues. The measured SWDGE wall (~2.9ns/descriptor agg) is
    the remaining floor.
"""

import math
import sys
from contextlib import ExitStack

for _p in ("/opt/trn_rl_repo",):
    if _p not in sys.path:
        sys.path.insert(0, _p)

import ml_dtypes
import numpy as np

import concourse.bacc as bacc
import concourse.bass as bass
import concourse.mybir as mybir
from concourse import library_config
from concourse.bass_utils import run_bass_kernel_spmd

F32 = mybir.dt.float32
BF16 = mybir.dt.bfloat16
I16 = mybir.dt.int16
NPBF16 = ml_dtypes.bfloat16
P = 128
COPY = mybir.ActivationFunctionType.Copy


class Cfg:
    def __init__(self, N, E, D=64, R=8, C=8, nchunk=5, ring=96):
        self.N, self.E, self.D, self.R, self.C = N, E, D, R, C
        assert N % C == 0
        self.NS = N // C                      # nodes per core
        self.G = math.ceil(self.NS / P)       # 128-node dst groups per core
        self.NS_PAD = self.G * P
        self.K = D * R // P                   # contraction chunks in phase A
        assert D * R % P == 0
        # AllGather chunks over groups
        base = self.G // nchunk
        sizes = [base + (1 if i < self.G % nchunk else 0) for i in range(nchunk)]
        self.chunks = []
        a = 0
        for s in sizes:
            self.chunks.append((a, a + s))
            a += s
        self.NCHUNK = nchunk
        # pair-table rows: chunk-major; chunk i holds rows for groups [a,b)
        # row(c, p2, g) = base_i + (c*64 + p2)*w_i + (g - a_i)
        self.tbase = np.zeros(nchunk + 1, np.int64)
        for i, (a, b) in enumerate(self.chunks):
            self.tbase[i + 1] = self.tbase[i] + C * 64 * (b - a)
        self.NTAB = int(self.tbase[-1])       # 25088 pair rows
        assert self.NTAB < 32768
        self.RING = ring


def plan_and_pack(cfg, x, edge_src, edge_dst, edge_weight, w_bases, w_rel):
    """Host preprocessing: sharding, tiling, schedule. Returns (plan, in_maps)."""
    C, NS, G, D, K = cfg.C, cfg.NS, cfg.G, cfg.D, cfg.K
    NCHUNK = cfg.NCHUNK

    es = edge_src.astype(np.int64)
    ed = edge_dst.astype(np.int64)
    ew = edge_weight.astype(np.float32)

    # dense weights: w[r] = sum_b w_rel[r,b] w_bases[b];  w_perm[(i,r), o]
    w = np.einsum("rb,bio->rio", w_rel.astype(np.float64),
                  w_bases.astype(np.float64)).astype(np.float32)
    w_perm = np.ascontiguousarray(w.transpose(1, 0, 2).reshape(D * cfg.R, D))
    wp = np.ascontiguousarray(
        w_perm.reshape(K, P, D).transpose(1, 0, 2).reshape(P, K * D)
    ).astype(NPBF16)

    # src -> (pair-table row, sigma, chunk)
    src_c, src_l = es // NS, es % NS
    g_s, p_s = src_l // P, src_l % P
    p2, sigma = p_s % 64, p_s // 64
    chunk_of = np.zeros(G, np.int64)
    for i, (a, b) in enumerate(cfg.chunks):
        chunk_of[a:b] = i
    ci = chunk_of[g_s]
    a_i = np.array([a for (a, b) in cfg.chunks])[ci]
    w_i = np.array([b - a for (a, b) in cfg.chunks])[ci]
    trow = cfg.tbase[ci] + (src_c * 64 + p2) * w_i + (g_s - a_i)

    dst_c, dst_l = ed // NS, ed % NS
    g_d, dloc = dst_l // P, dst_l % P

    # per-core edge ordering: (dst group, sigma), then src chunk
    key = (g_d * 2 + sigma)
    per_core = []
    cnt = np.zeros((C, G * 2), np.int64)
    for c in range(C):
        m = np.where(dst_c == c)[0]
        order = m[np.lexsort((ci[m], key[m]))]
        per_core.append(order)
        b = np.bincount(key[m], minlength=G * 2)
        cnt[c] = b
    bounds = np.zeros((C, G * 2 + 1), np.int64)
    bounds[:, 1:] = np.cumsum(cnt, axis=1)

    # SPMD-uniform tile counts per (g, sigma)
    Tgs = np.maximum(0, np.ceil(cnt.max(axis=0) / P).astype(np.int64))  # [G*2]
    # tile ready-epoch = max over cores of chunk of last edge in tile
    tiles = []   # (g, sigma, t, epoch)
    for gs in range(G * 2):
        g, sg = gs // 2, gs % 2
        for t in range(int(Tgs[gs])):
            ep = 0
            for c in range(C):
                n = cnt[c, gs]
                if n <= t * P:
                    continue
                last = min(n, (t + 1) * P) - 1
                e_idx = per_core[c][bounds[c, gs] + last]
                ep = max(ep, int(ci[e_idx]))
            tiles.append((g, sg, t, ep))
    # global order: by (epoch, g, sigma, t)
    tiles.sort(key=lambda z: (z[3], z[0], z[1], z[2]))
    TT = len(tiles)
    pos_of = {}
    for pos, (g, sg, t, ep) in enumerate(tiles):
        pos_of[(g, sg, t)] = pos

    # runs: consecutive tiles with same (g, sigma) (epoch-sorted so same ep)
    runs = []   # dict(g, sigma, pos, nt, epoch)
    for pos, (g, sg, t, ep) in enumerate(tiles):
        if runs and runs[-1]["g"] == g and runs[-1]["sigma"] == sg \
                and runs[-1]["pos"] + runs[-1]["nt"] == pos \
                and runs[-1]["epoch"] == ep:
            runs[-1]["nt"] += 1
        else:
            runs.append(dict(g=g, sigma=sg, pos=pos, nt=1, epoch=ep))

    # explicit ring positions: runs never straddle the ring boundary
    RING = cfg.RING
    rpos = 0
    for r in runs:
        if rpos + r["nt"] > RING:
            rpos = 0
        r["posr"] = rpos
        rpos += r["nt"]

    # calls: pack runs per epoch into ~4 calls; calls are ring-contiguous
    calls = []  # dict(epoch, pos, nt, posr, queue, qidx, guard)
    qn = 0
    qidx = [0, 0, 0, 0]
    for ep in range(NCHUNK):
        ep_runs = [r for r in runs if r["epoch"] == ep]
        if not ep_runs:
            continue
        tot = sum(r["nt"] for r in ep_runs)
        tgt = max(1, math.ceil(tot / 8))
        cur = None
        for r in ep_runs:
            contig = cur is not None and \
                r["posr"] == cur["posr"] + cur["nt"]
            if cur is None or cur["nt"] >= tgt or not contig:
                if cur is not None:
                    calls.append(cur)
                cur = dict(epoch=ep, pos=r["pos"], nt=0, posr=r["posr"],
                           queue=qn % 4, qidx=qidx[qn % 4])
                qidx[qn % 4] += 1
                qn += 1
            cur["nt"] += r["nt"]
            r["call"] = cur
        calls.append(cur)
    # indices: s_mm counts RUNS (inc 1 each), s_dve counts DVE OPS
    for ridx, r in enumerate(runs):
        r["ridx"] = ridx
    for kidx, ccall in enumerate(calls):
        ccall["kidx"] = kidx
    for r in runs:
        r["dve_target"] = 2 * (r["ridx"] + 1)
    # ring-reuse guards in run-count units
    slot_need = np.zeros(RING, np.int64)
    for ccall in calls:
        a, b = ccall["posr"], ccall["posr"] + ccall["nt"]
        ccall["guard"] = int(slot_need[a:b].max())
    # two passes: guards above read pre-update state per call in order
    slot_need[:] = 0
    for ccall in calls:
        a, b = ccall["posr"], ccall["posr"] + ccall["nt"]
        ccall["guard"] = int(slot_need[a:b].max())
        for r in runs:
            if r.get("call") is ccall:
                slot_need[r["posr"]: r["posr"] + r["nt"]] = r["ridx"] + 1

    # group first/last tiles + completion order
    first_pos = np.full(G, TT, np.int64)
    last_pos = np.full(G, -1, np.int64)
    for pos, (g, sg, t, ep) in enumerate(tiles):
        first_pos[g] = min(first_pos[g], pos)
        last_pos[g] = max(last_pos[g], pos)
    comp_order = sorted(range(G), key=lambda g: last_pos[g])
    slot_of = np.zeros(G, np.int64)
    for j, g in enumerate(comp_order):
        slot_of[g] = j
    # s_mm is incremented by nt at end of each run; cumulative tiles through
    # the run containing pos:


    run_end = np.zeros(TT, np.int64)   # for tile pos: runs done incl its run
    for r in runs:
        run_end[r["pos"]: r["pos"] + r["nt"]] = r["ridx"] + 1

    # per-core data arrays
    in_maps = []
    for c in range(C):
        gidx = np.zeros((P, 8 * TT), np.int16)
        dst_a = np.full((P, TT), 999.0, np.float32)
        wgt_a = np.zeros((P, TT), np.float32)
        ord_c = per_core[c]
        for gs in range(G * 2):
            g, sg = gs // 2, gs % 2
            lo, hi = bounds[c, gs], bounds[c, gs + 1]
            n = hi - lo
            for t in range(int(Tgs[gs])):
                pos = pos_of[(g, sg, t)]
                s0, s1 = t * P, min(n, (t + 1) * P)
                L = max(0, s1 - s0)
                vals = np.zeros(P, np.int64)
                if L > 0:
                    eidx = ord_c[lo + s0: lo + s1]
                    vals[:L] = trow[eidx]
                    dst_a[:L, pos] = dloc[eidx]
                    wgt_a[:L, pos] = ew[eidx]
                gidx[:, 8 * pos: 8 * (pos + 1)] = np.tile(
                    vals.astype(np.int16).reshape(8, 16).T, (8, 1))
        xs = x[c * NS:(c + 1) * NS].reshape(NS, D * cfg.R)
        xT = np.zeros((D * cfg.R, cfg.NS_PAD), NPBF16)
        xT[:, :NS] = xs.T.astype(NPBF16)
        iota = np.broadcast_to(np.arange(P, dtype=np.float32),
                               (P, P)).astype(NPBF16).copy()
        in_maps.append({
            "xT": xT, "wp": wp.copy(), "gidx": gidx,
            "dstloc": dst_a.astype(NPBF16), "wgt": wgt_a.astype(NPBF16),
            "iota": iota,
        })

    plan = {"tiles": tiles, "runs": runs, "calls": calls, "TT": TT,
            "first_pos": first_pos.tolist(), "last_pos": last_pos.tolist(),
            "comp_order": comp_order, "slot_of": slot_of.tolist(),
            "run_end": run_end.tolist()}
    return plan, in_maps


def build_nc(cfg, plan):
    C, G, D, K, NCHUNK = cfg.C, cfg.G, cfg.D, cfg.K, cfg.NCHUNK
    NS_PAD, TT, RING = cfg.NS_PAD, plan["TT"], cfg.RING
    runs, calls, tiles = plan["runs"], plan["calls"], plan["tiles"]
    first_pos, last_pos = plan["first_pos"], plan["last_pos"]
    comp_order, run_end = plan["comp_order"], plan["run_end"]
    chunks, tbase = cfg.chunks, cfg.tbase

    nc = bacc.Bacc("TRN2", num_swdge_queues=4,
               detect_race_conditions=False)

    xT_d = nc.declare_dram_parameter("xT", [K * P, NS_PAD], BF16, isOutput=False)
    wp_d = nc.declare_dram_parameter("wp", [P, K * D], BF16, isOutput=False)
    gidx_d = nc.declare_dram_parameter("gidx", [P, 8 * TT], I16, isOutput=False)
    dst_d = nc.declare_dram_parameter("dstloc", [P, TT], BF16, isOutput=False)
    wgt_d = nc.declare_dram_parameter("wgt", [P, TT], BF16, isOutput=False)
    iota_d = nc.declare_dram_parameter("iota", [P, P], BF16, isOutput=False)
    out_d = nc.declare_dram_parameter("out", [P, G * D], F32, isOutput=True)

    # pair table: per chunk i (groups [a,b), w=b-a): [64, w, 2, 64] bf16
    y_own = nc.dram_tensor("y_own", [64 * G * 2 * D], BF16)
    y_all = nc.dram_tensor("y_all", [C * 64 * G * 2 * D], BF16,
                           addr_space="Shared")
    obase = [int(tbase[i]) // C * 2 * D for i in range(NCHUNK + 1)]
    abase = [int(tbase[i]) * 2 * D for i in range(NCHUNK + 1)]

    with ExitStack() as top:
        sem = top.enter_context
        s_wp = sem(nc.semaphore("s_wp"))
        s_xt = [sem(nc.semaphore(f"s_xt{i}")) for i in range(NCHUNK)]
        s_meta = sem(nc.semaphore("s_meta"))
        s_mmA = sem(nc.semaphore("s_mmA"))
        s_yA = sem(nc.semaphore("s_yA"))
        s_ydma = [sem(nc.semaphore(f"s_ydma{i}")) for i in range(NCHUNK)]
        s_cc = sem(nc.semaphore("s_cc"))
        s_g = [sem(nc.semaphore(f"s_g{q}")) for q in range(4)]
        s_dve = sem(nc.semaphore("s_dve"))
        s_mm = sem(nc.semaphore("s_mm"))
        s_po = sem(nc.semaphore("s_po"))
        s_od = sem(nc.semaphore("s_od"))
        s_ms = sem(nc.semaphore("s_ms"))
        s_pz = sem(nc.semaphore("s_pz"))

        sb = top.enter_context
        gidx_sb = sb(nc.sbuf_tensor("gidx_sb", [P, 8 * TT], I16))
        dst_sb = sb(nc.sbuf_tensor("dst_sb", [P, TT], BF16))
        wgt_sb = sb(nc.sbuf_tensor("wgt_sb", [P, TT], BF16))
        iota_sb = sb(nc.sbuf_tensor("iota_sb", [P, P], BF16))
        xT_sb = sb(nc.sbuf_tensor("xT_sb", [P, K, NS_PAD], BF16))
        wp_sb = sb(nc.sbuf_tensor("wp_sb", [P, K * D], BF16))
        y_sb = sb(nc.sbuf_tensor("y_sb", [P, G, D], BF16))
        out_sb = sb(nc.sbuf_tensor("out_sb", [P, G, D], F32))
        gbuf = sb(nc.sbuf_tensor("gbuf", [P, RING, 2 * D], BF16))
        mbuf = sb(nc.sbuf_tensor("mbuf", [P, RING, P], BF16))
        msg = sb(nc.sbuf_tensor("msg", [P, RING, D], BF16))
        psA = sb(nc.psum_tensor("psA", [P, 4, D], F32))
        psC = sb(nc.psum_tensor("psC", [P, G, D], F32))

        def psc(g):
            return psC[:, g, :]

        batches = []      # (gstart, gend) — phase A groups, <=4, chunk-pure
        for (a, b) in chunks:
            g0 = a
            while g0 < b:
                batches.append((g0, min(b, g0 + 4)))
                g0 += 4
        nb_thru = []      # cumulative batch count through each chunk
        nb = 0
        for (a, b) in chunks:
            nb += math.ceil((b - a) / 4)
            nb_thru.append(nb)
        y_rows = y_all.rearrange("(q d) -> q d", d=2 * D)
        blockC = top.enter_context(nc.Block())

        @blockC.sync
        def _(sync):
            sync.dma_start(out=wp_sb[:], in_=wp_d[:]).then_inc(s_wp, 16)
            xr = xT_d.rearrange("(k p) n -> p k n", p=P)
            for i, (a, b) in enumerate(chunks):
                sync.dma_start(
                    out=xT_sb[:, :, a * P:b * P], in_=xr[:, :, a * P:b * P],
                ).then_inc(s_xt[i], 16)
            sync.dma_start(out=gidx_sb[:], in_=gidx_d[:]).then_inc(s_meta, 16)
            sync.dma_start(out=dst_sb[:], in_=dst_d[:]).then_inc(s_meta, 16)
            sync.dma_start(out=wgt_sb[:], in_=wgt_d[:]).then_inc(s_meta, 16)
            sync.dma_start(out=iota_sb[:], in_=iota_d[:]).then_inc(s_meta, 16)
            # y pair-table writes per chunk: two partition-halves
            for i, (a, b) in enumerate(chunks):
                w = b - a
                reg = y_own[obase[i]:obase[i + 1]].rearrange(
                    "(p w s d) -> p w s d", p=64, w=w, s=2, d=D)
                sync.wait_ge(s_yA, nb_thru[i])
                sync.dma_start(
                    out=reg[:, :, 0, :], in_=y_sb[0:64, a:b, :],
                ).then_inc(s_ydma[i], 16)
                sync.dma_start(
                    out=reg[:, :, 1, :], in_=y_sb[64:128, a:b, :],
                ).then_inc(s_ydma[i], 16)
            # output DMAs in completion-slot batches
            nod = 0
            step = math.ceil(G / NCHUNK)
            done = 0
            for a in range(0, G, step):
                b = min(G, a + step)
                # wait until the b-th completed group's copy is done
                sync.wait_ge(s_po, b)
                sync.dma_start(
                    out=out_d[:, a * D:b * D], in_=out_sb[:, a:b, :],
                ).then_inc(s_od, 16)
                nod += 16
            sync.wait_ge(s_od, nod)

        @blockC.tensor
        def _(tensor):
            tensor.wait_ge(s_wp, 16)
            ci = 0
            for bi, (ga, gb) in enumerate(batches):
                while ga >= chunks[ci][1]:
                    ci += 1
                if ga == chunks[ci][0]:
                    tensor.wait_ge(s_xt[ci], 16)
                if bi >= 1:
                    tensor.wait_ge(s_yA, bi)
                for j, nt in enumerate(range(ga, gb)):
                    for k in range(K):
                        mm = tensor.matmul(
                            psA[:, j, :],
                            xT_sb[:, k, nt * P:(nt + 1) * P],
                            wp_sb[:, k * D:(k + 1) * D],
                            start=(j == 0 and k == 0),
                            stop=(nt == gb - 1 and k == K - 1),
                        )
                mm.then_inc(s_mmA, 1)
            # phase C
            tensor.wait_ge(s_pz, 1)
            for r in runs:
                g, pos, nt = r["g"], r["pos"], r["nt"]
                tensor.wait_ge(s_dve, r["dve_target"])
                for i in range(nt):
                    posr = r["posr"] + i
                    mm = tensor.matmul(
                        psc(g),
                        mbuf[:, posr, :],
                        msg[:, posr, :],
                        start=False, stop=False, skip_group_check=True,
                    )
                mm.then_inc(s_mm, 1)

        @blockC.scalar
        def _(scalar):
            # phase A: per-batch psum -> bf16 y_sb copy
            for bi, (ga, gb) in enumerate(batches):
                scalar.wait_ge(s_mmA, bi + 1)
                scalar.activation(
                    out=y_sb[:, ga:gb, :], in_=psA[:, 0:gb - ga, :],
                    func=COPY,
                ).then_inc(s_yA, 1)
            # phase C: completed-group copies, in completion order
            for j, g in enumerate(comp_order):
                scalar.wait_ge(s_mm, int(run_end[last_pos[g]]))
                scalar.copy(out_sb[:, j, :], psc(g)).then_inc(s_po, 1)

        @blockC.vector
        def _(vector):
            vector.memset(psC[:], 0.0).then_inc(s_pz, 1)
            vector.memset(gbuf[:], 0.0).then_inc(s_ms, 1)
            vector.wait_ge(s_meta, 64)
            cur_call = None
            for r in runs:
                pos, nt, posr, sg2 = r["pos"], r["nt"], r["posr"], r["sigma"]
                cc = r["call"]
                if cc is not cur_call:
                    vector.wait_ge(s_g[cc["queue"]], 16 * (cc["qidx"] + 1))
                    cur_call = cc
                dstb = dst_sb[:, pos:pos + nt].unsqueeze(2).to_broadcast(
                    [P, nt, P])
                iotb = iota_sb[:, :].unsqueeze(1).to_broadcast([P, nt, P])
                vector.tensor_tensor(
                    out=mbuf[:, posr:posr + nt, :], in0=dstb, in1=iotb,
                    op=mybir.AluOpType.is_equal,
                ).then_inc(s_dve, 1)
                wgtb = wgt_sb[:, pos:pos + nt].unsqueeze(2).to_broadcast(
                    [P, nt, D])
                vector.tensor_tensor(
                    out=msg[:, posr:posr + nt, :],
                    in0=gbuf[:, posr:posr + nt, sg2 * D:(sg2 + 1) * D],
                    in1=wgtb,
                    op=mybir.AluOpType.mult,
                ).then_inc(s_dve, 1)

        @blockC.gpsimd
        def _(gpsimd):
            gpsimd.load_library(library_config.mlp)

            def ag(i):
                gpsimd.wait_ge(s_ydma[i], 32)
                gpsimd.collective_compute(
                    "AllGather",
                    mybir.AluOpType.bypass,
                    replica_groups=[list(range(C))],
                    ins=[y_own[obase[i]:obase[i + 1]].opt()],
                    outs=[y_all[abase[i]:abase[i + 1]].opt()],
                ).then_inc(s_cc)

            def do_call(ccall):
                pos, nt = ccall["pos"], ccall["nt"]
                gpsimd.wait_ge(s_cc, ccall["epoch"] + 1)
                if ccall["guard"] > 0:
                    gpsimd.wait_ge(s_mm, ccall["guard"])
                posr = ccall["posr"]
                gpsimd.dma_gather(
                    gbuf[:, posr:posr + nt, :],
                    y_rows[0:int(tbase[ccall["epoch"] + 1]), :],
                    gidx_sb[:, 8 * pos: 8 * (pos + nt)],
                    nt * P, nt * P, 2 * D,
                    single_packet=False, queue_num=ccall["queue"],
                ).then_inc(s_g[ccall["queue"]], 16)

            by_ep = [[cl for cl in calls if cl["epoch"] == ep]
                     for ep in range(NCHUNK)]
            gpsimd.wait_ge(s_ms, 1)
            ag(0)
            if NCHUNK > 1:
                ag(1)
            for ep in range(NCHUNK):
                for ccall in by_ep[ep]:
                    do_call(ccall)
                if ep + 2 < NCHUNK:
                    ag(ep + 2)

    nc.compile()
    return nc


def _assemble(cfg, plan, outs):
    D, G, NS = cfg.D, cfg.G, cfg.NS
    comp_order = plan["comp_order"]
    full = np.empty((cfg.N, D), np.float32)
    for c in range(cfg.C):
        o = outs[c]["out"].reshape(P, G, D)
        per_g = np.empty((G, P, D), np.float32)
        for j, g in enumerate(comp_order):
            per_g[g] = o[:, j, :]
        flat = per_g.transpose(0, 1, 2).reshape(cfg.NS_PAD, D)
        full[c * NS:(c + 1) * NS] = flat[:NS]
    return full


def gnn_kernel(x, edge_src, edge_dst, edge_weight, w_bases, w_rel,
               cfg=None, trace=False):
    if cfg is None:
        cfg = Cfg(N=50000, E=800000)
    plan, in_maps = plan_and_pack(cfg, np.asarray(x), np.asarray(edge_src),
                                  np.asarray(edge_dst), np.asarray(edge_weight),
                                  np.asarray(w_bases), np.asarray(w_rel))
    nc = build_nc(cfg, plan)
    res = run_bass_kernel_spmd(nc, in_maps, list(range(cfg.C)), trace=trace)
    return _assemble(cfg, plan, res.results), res


def kernel(x, edge_src, edge_dst, edge_weight, w_bases, w_rel):
    """Full inputs in, full output out. Shards across 8 NeuronCores inside."""
    full, _ = gnn_kernel(x, edge_src, edge_dst, edge_weight, w_bases, w_rel)
    return full


# revision 32
# speedup vs baseline: 1.4186x; 1.1156x over previous
"""RGCN basis-decomposed message passing on 8 TRN2 NeuronCores (v3).

Math: out = A @ y,  y = x_flat @ w_perm  (linearity collapse: the edge
aggregation is relation-independent once y is precomputed).

v3 design (vs v2 baseline at 524us):
  - y is stored as bf16 NODE-PAIRS: table row q holds [y[node p] | y[node
    p+64]] of one 128-node group -> 256B gather elements, 25088 rows total
    (fits a single int16 index window). AllGather traffic halved vs f32.
  - The y AllGather runs in 5 group-chunks pipelined behind phase A, so
    the first edge gathers start at ~20us instead of ~100us.
  - Edge tiles are (dst-group, sigma)-pure where sigma = src partition
    // 64 selects the pair half: the half-select is a free AP slice of
    the gathered buffer. Edge weights are applied by one batched DVE
    multiply per run; the scatter one-hot by one batched DVE is_equal.
    The Scalar engine only copies finished PSUM accumulators out.
  - All 49 dst-group accumulators are PSUM-resident simultaneously
    (7 banks x 8 slices + 1 bank for phase A), so matmuls accumulate
    per-group across AllGather epochs with no barrier.
  - Gathers are merged into ~4 large dma_gather calls per epoch over the
    4 SWDGE queues. The measured SWDGE wall (~2.9ns/descriptor agg) is
    the remaining floor.
"""

import math
import sys
from contextlib import ExitStack

for _p in ("/opt/trn_rl_repo",):
    if _p not in sys.path:
        sys.path.insert(0, _p)

import ml_dtypes
import numpy as np

import concourse.bacc as bacc
import concourse.bass as bass
import concourse.mybir as mybir
from concourse import library_config
from concourse.bass_utils import run_bass_kernel_spmd

F32 = mybir.dt.float32
BF16 = mybir.dt.bfloat16
I16 = mybir.dt.int16
NPBF16 = ml_dtypes.bfloat16
P = 128
COPY = mybir.ActivationFunctionType.Copy


class Cfg:
    def __init__(self, N, E, D=64, R=8, C=8, nchunk=5, ring=128):
        self.N, self.E, self.D, self.R, self.C = N, E, D, R, C
        assert N % C == 0
        self.NS = N // C                      # nodes per core
        self.G = math.ceil(self.NS / P)       # 128-node dst groups per core
        self.NS_PAD = self.G * P
        self.K = D * R // P                   # contraction chunks in phase A
        assert D * R % P == 0
        # AllGather chunks over groups
        base = self.G // nchunk
        sizes = [base + (1 if i < self.G % nchunk else 0) for i in range(nchunk)]
        self.chunks = []
        a = 0
        for s in sizes:
            self.chunks.append((a, a + s))
            a += s
        self.NCHUNK = nchunk
        # pair-table rows: chunk-major; chunk i holds rows for groups [a,b)
        # row(c, p2, g) = base_i + (c*64 + p2)*w_i + (g - a_i)
        self.tbase = np.zeros(nchunk + 1, np.int64)
        for i, (a, b) in enumerate(self.chunks):
            self.tbase[i + 1] = self.tbase[i] + C * 64 * (b - a)
        self.NTAB = int(self.tbase[-1])       # 25088 pair rows
        assert self.NTAB < 32768
        self.RING = ring


def plan_and_pack(cfg, x, edge_src, edge_dst, edge_weight, w_bases, w_rel):
    """Host preprocessing: sharding, tiling, schedule. Returns (plan, in_maps)."""
    C, NS, G, D, K = cfg.C, cfg.NS, cfg.G, cfg.D, cfg.K
    NCHUNK = cfg.NCHUNK

    es = edge_src.astype(np.int64)
    ed = edge_dst.astype(np.int64)
    ew = edge_weight.astype(np.float32)

    # dense weights: w[r] = sum_b w_rel[r,b] w_bases[b];  w_perm[(i,r), o]
    w = np.einsum("rb,bio->rio", w_rel.astype(np.float64),
                  w_bases.astype(np.float64)).astype(np.float32)
    w_perm = np.ascontiguousarray(w.transpose(1, 0, 2).reshape(D * cfg.R, D))
    wp = np.ascontiguousarray(
        w_perm.reshape(K, P, D).transpose(1, 0, 2).reshape(P, K * D)
    ).astype(NPBF16)

    # src -> (pair-table row, sigma, chunk)
    src_c, src_l = es // NS, es % NS
    g_s, p_s = src_l // P, src_l % P
    p2, sigma = p_s % 64, p_s // 64
    chunk_of = np.zeros(G, np.int64)
    for i, (a, b) in enumerate(cfg.chunks):
        chunk_of[a:b] = i
    ci = chunk_of[g_s]
    a_i = np.array([a for (a, b) in cfg.chunks])[ci]
    w_i = np.array([b - a for (a, b) in cfg.chunks])[ci]
    trow = cfg.tbase[ci] + (src_c * 64 + p2) * w_i + (g_s - a_i)

    dst_c, dst_l = ed // NS, ed % NS
    g_d, dloc = dst_l // P, dst_l % P

    # per-core edge ordering: (dst group, sigma), then src chunk
    key = (g_d * 2 + sigma)
    per_core = []
    cnt = np.zeros((C, G * 2), np.int64)
    for c in range(C):
        m = np.where(dst_c == c)[0]
        order = m[np.lexsort((ci[m], key[m]))]
        per_core.append(order)
        b = np.bincount(key[m], minlength=G * 2)
        cnt[c] = b
    bounds = np.zeros((C, G * 2 + 1), np.int64)
    bounds[:, 1:] = np.cumsum(cnt, axis=1)

    # SPMD-uniform tile counts per (g, sigma)
    Tgs = np.maximum(0, np.ceil(cnt.max(axis=0) / P).astype(np.int64))  # [G*2]
    # tile ready-epoch = max over cores of chunk of last edge in tile
    tiles = []   # (g, sigma, t, epoch)
    for gs in range(G * 2):
        g, sg = gs // 2, gs % 2
        for t in range(int(Tgs[gs])):
            ep = 0
            for c in range(C):
                n = cnt[c, gs]
                if n <= t * P:
                    continue
                last = min(n, (t + 1) * P) - 1
                e_idx = per_core[c][bounds[c, gs] + last]
                ep = max(ep, int(ci[e_idx]))
            tiles.append((g, sg, t, ep))
    # global order: by (epoch, g, sigma, t)
    tiles.sort(key=lambda z: (z[3], z[0], z[1], z[2]))
    TT = len(tiles)
    pos_of = {}
    for pos, (g, sg, t, ep) in enumerate(tiles):
        pos_of[(g, sg, t)] = pos

    # runs: consecutive tiles with same (g, sigma) (epoch-sorted so same ep)
    runs = []   # dict(g, sigma, pos, nt, epoch)
    for pos, (g, sg, t, ep) in enumerate(tiles):
        if runs and runs[-1]["g"] == g and runs[-1]["sigma"] == sg \
                and runs[-1]["pos"] + runs[-1]["nt"] == pos \
                and runs[-1]["epoch"] == ep:
            runs[-1]["nt"] += 1
        else:
            runs.append(dict(g=g, sigma=sg, pos=pos, nt=1, epoch=ep))

    # explicit ring positions: runs never straddle the ring boundary
    RING = cfg.RING
    rpos = 0
    for r in runs:
        if rpos + r["nt"] > RING:
            rpos = 0
        r["posr"] = rpos
        rpos += r["nt"]

    # calls: pack runs per epoch into ~4 calls; calls are ring-contiguous
    calls = []  # dict(epoch, pos, nt, posr, queue, qidx, guard)
    qn = 0
    qidx = [0, 0, 0, 0]
    for ep in range(NCHUNK):
        ep_runs = [r for r in runs if r["epoch"] == ep]
        if not ep_runs:
            continue
        tot = sum(r["nt"] for r in ep_runs)
        tgt = max(1, math.ceil(tot / 8))
        cur = None
        for r in ep_runs:
            contig = cur is not None and \
                r["posr"] == cur["posr"] + cur["nt"]
            if cur is None or cur["nt"] >= tgt or not contig:
                if cur is not None:
                    calls.append(cur)
                cur = dict(epoch=ep, pos=r["pos"], nt=0, posr=r["posr"],
                           queue=qn % 4, qidx=qidx[qn % 4])
                qidx[qn % 4] += 1
                qn += 1
            cur["nt"] += r["nt"]
            r["call"] = cur
        calls.append(cur)
    qtot = [0, 0, 0, 0]
    for ccall in calls:
        qtot[ccall["queue"]] += 1
    for ccall in calls:
        ccall["qwait"] = 16 * min(ccall["qidx"] + 2, qtot[ccall["queue"]])

    # indices: s_mm counts RUNS (inc 1 each), s_dve counts DVE OPS
    for ridx, r in enumerate(runs):
        r["ridx"] = ridx
    for kidx, ccall in enumerate(calls):
        ccall["kidx"] = kidx
    for r in runs:
        r["dve_target"] = 2 * (r["ridx"] + 1)
    # ring-reuse guards in run-count units
    slot_need = np.zeros(RING, np.int64)
    for ccall in calls:
        a, b = ccall["posr"], ccall["posr"] + ccall["nt"]
        ccall["guard"] = int(slot_need[a:b].max())
    # two passes: guards above read pre-update state per call in order
    slot_need[:] = 0
    for ccall in calls:
        a, b = ccall["posr"], ccall["posr"] + ccall["nt"]
        ccall["guard"] = int(slot_need[a:b].max())
        for r in runs:
            if r.get("call") is ccall:
                slot_need[r["posr"]: r["posr"] + r["nt"]] = r["ridx"] + 1

    # group first/last tiles + completion order
    first_pos = np.full(G, TT, np.int64)
    last_pos = np.full(G, -1, np.int64)
    for pos, (g, sg, t, ep) in enumerate(tiles):
        first_pos[g] = min(first_pos[g], pos)
        last_pos[g] = max(last_pos[g], pos)
    comp_order = sorted(range(G), key=lambda g: last_pos[g])
    slot_of = np.zeros(G, np.int64)
    for j, g in enumerate(comp_order):
        slot_of[g] = j
    # s_mm is incremented by nt at end of each run; cumulative tiles through
    # the run containing pos:


    run_end = np.zeros(TT, np.int64)   # for tile pos: runs done incl its run
    for r in runs:
        run_end[r["pos"]: r["pos"] + r["nt"]] = r["ridx"] + 1

    # per-core data arrays
    in_maps = []
    for c in range(C):
        gidx = np.zeros((P, 8 * TT), np.int16)
        dst_a = np.full((P, TT), 999.0, np.float32)
        wgt_a = np.zeros((P, TT), np.float32)
        ord_c = per_core[c]
        for gs in range(G * 2):
            g, sg = gs // 2, gs % 2
            lo, hi = bounds[c, gs], bounds[c, gs + 1]
            n = hi - lo
            for t in range(int(Tgs[gs])):
                pos = pos_of[(g, sg, t)]
                s0, s1 = t * P, min(n, (t + 1) * P)
                L = max(0, s1 - s0)
                vals = np.zeros(P, np.int64)
                if L > 0:
                    eidx = ord_c[lo + s0: lo + s1]
                    vals[:L] = trow[eidx]
                    dst_a[:L, pos] = dloc[eidx]
                    wgt_a[:L, pos] = ew[eidx]
                gidx[:, 8 * pos: 8 * (pos + 1)] = np.tile(
                    vals.astype(np.int16).reshape(8, 16).T, (8, 1))
        xs = x[c * NS:(c + 1) * NS].reshape(NS, D * cfg.R)
        xT = np.zeros((D * cfg.R, cfg.NS_PAD), NPBF16)
        xT[:, :NS] = xs.T.astype(NPBF16)
        iota = np.broadcast_to(np.arange(P, dtype=np.float32),
                               (P, P)).astype(NPBF16).copy()
        in_maps.append({
            "xT": xT, "wp": wp.copy(), "gidx": gidx,
            "dstloc": dst_a.astype(NPBF16), "wgt": wgt_a.astype(NPBF16),
            "iota": iota,
        })

    plan = {"tiles": tiles, "runs": runs, "calls": calls, "TT": TT,
            "first_pos": first_pos.tolist(), "last_pos": last_pos.tolist(),
            "comp_order": comp_order, "slot_of": slot_of.tolist(),
            "run_end": run_end.tolist()}
    return plan, in_maps


def build_nc(cfg, plan):
    C, G, D, K, NCHUNK = cfg.C, cfg.G, cfg.D, cfg.K, cfg.NCHUNK
    NS_PAD, TT, RING = cfg.NS_PAD, plan["TT"], cfg.RING
    runs, calls, tiles = plan["runs"], plan["calls"], plan["tiles"]
    first_pos, last_pos = plan["first_pos"], plan["last_pos"]
    comp_order, run_end = plan["comp_order"], plan["run_end"]
    chunks, tbase = cfg.chunks, cfg.tbase

    nc = bacc.Bacc("TRN2", num_swdge_queues=4,
               detect_race_conditions=False)

    xT_d = nc.declare_dram_parameter("xT", [K * P, NS_PAD], BF16, isOutput=False)
    wp_d = nc.declare_dram_parameter("wp", [P, K * D], BF16, isOutput=False)
    gidx_d = nc.declare_dram_parameter("gidx", [P, 8 * TT], I16, isOutput=False)
    dst_d = nc.declare_dram_parameter("dstloc", [P, TT], BF16, isOutput=False)
    wgt_d = nc.declare_dram_parameter("wgt", [P, TT], BF16, isOutput=False)
    iota_d = nc.declare_dram_parameter("iota", [P, P], BF16, isOutput=False)
    out_d = nc.declare_dram_parameter("out", [P, G * D], F32, isOutput=True)

    # pair table: per chunk i (groups [a,b), w=b-a): [64, w, 2, 64] bf16
    y_own = nc.dram_tensor("y_own", [64 * G * 2 * D], BF16)
    y_all = nc.dram_tensor("y_all", [C * 64 * G * 2 * D], BF16,
                           addr_space="Shared")
    obase = [int(tbase[i]) // C * 2 * D for i in range(NCHUNK + 1)]
    abase = [int(tbase[i]) * 2 * D for i in range(NCHUNK + 1)]

    with ExitStack() as top:
        sem = top.enter_context
        s_wp = sem(nc.semaphore("s_wp"))
        s_xt = [sem(nc.semaphore(f"s_xt{i}")) for i in range(NCHUNK)]
        s_meta = sem(nc.semaphore("s_meta"))
        s_mmA = sem(nc.semaphore("s_mmA"))
        s_yA = sem(nc.semaphore("s_yA"))
        s_ydma = [sem(nc.semaphore(f"s_ydma{i}")) for i in range(NCHUNK)]
        s_cc = sem(nc.semaphore("s_cc"))
        s_g = [sem(nc.semaphore(f"s_g{q}")) for q in range(4)]
        s_dve = sem(nc.semaphore("s_dve"))
        s_mm = sem(nc.semaphore("s_mm"))
        s_po = sem(nc.semaphore("s_po"))
        s_od = sem(nc.semaphore("s_od"))
        s_ms = sem(nc.semaphore("s_ms"))
        s_pz = sem(nc.semaphore("s_pz"))

        sb = top.enter_context
        gidx_sb = sb(nc.sbuf_tensor("gidx_sb", [P, 8 * TT], I16))
        dst_sb = sb(nc.sbuf_tensor("dst_sb", [P, TT], BF16))
        wgt_sb = sb(nc.sbuf_tensor("wgt_sb", [P, TT], BF16))
        iota_sb = sb(nc.sbuf_tensor("iota_sb", [P, P], BF16))
        xT_sb = sb(nc.sbuf_tensor("xT_sb", [P, K, NS_PAD], BF16))
        wp_sb = sb(nc.sbuf_tensor("wp_sb", [P, K * D], BF16))
        y_sb = sb(nc.sbuf_tensor("y_sb", [P, G, D], BF16))
        out_sb = sb(nc.sbuf_tensor("out_sb", [P, G, D], F32))
        gbuf = sb(nc.sbuf_tensor("gbuf", [P, RING, 2 * D], BF16))
        mbuf = sb(nc.sbuf_tensor("mbuf", [P, RING, P], BF16))
        msg = sb(nc.sbuf_tensor("msg", [P, RING, D], BF16))
        psA = sb(nc.psum_tensor("psA", [P, 4, D], F32))
        psC = sb(nc.psum_tensor("psC", [P, G, D], F32))

        def psc(g):
            return psC[:, g, :]

        batches = []      # (gstart, gend) — phase A groups, <=4, chunk-pure
        for (a, b) in chunks:
            g0 = a
            while g0 < b:
                batches.append((g0, min(b, g0 + 4)))
                g0 += 4
        nb_thru = []      # cumulative batch count through each chunk
        nb = 0
        for (a, b) in chunks:
            nb += math.ceil((b - a) / 4)
            nb_thru.append(nb)
        y_rows = y_all.rearrange("(q d) -> q d", d=2 * D)
        blockC = top.enter_context(nc.Block())

        @blockC.sync
        def _(sync):
            sync.dma_start(out=wp_sb[:], in_=wp_d[:]).then_inc(s_wp, 16)
            xr = xT_d.rearrange("(k p) n -> p k n", p=P)
            for i, (a, b) in enumerate(chunks):
                sync.dma_start(
                    out=xT_sb[:, :, a * P:b * P], in_=xr[:, :, a * P:b * P],
                ).then_inc(s_xt[i], 16)
            sync.dma_start(out=gidx_sb[:], in_=gidx_d[:]).then_inc(s_meta, 16)
            sync.dma_start(out=dst_sb[:], in_=dst_d[:]).then_inc(s_meta, 16)
            sync.dma_start(out=wgt_sb[:], in_=wgt_d[:]).then_inc(s_meta, 16)
            sync.dma_start(out=iota_sb[:], in_=iota_d[:]).then_inc(s_meta, 16)
            # y pair-table writes per chunk: two partition-halves
            for i, (a, b) in enumerate(chunks):
                w = b - a
                reg = y_own[obase[i]:obase[i + 1]].rearrange(
                    "(p w s d) -> p w s d", p=64, w=w, s=2, d=D)
                sync.wait_ge(s_yA, nb_thru[i])
                sync.dma_start(
                    out=reg[:, :, 0, :], in_=y_sb[0:64, a:b, :],
                ).then_inc(s_ydma[i], 16)
                sync.dma_start(
                    out=reg[:, :, 1, :], in_=y_sb[64:128, a:b, :],
                ).then_inc(s_ydma[i], 16)
            # output DMAs in completion-slot batches
            nod = 0
            step = math.ceil(G / NCHUNK)
            done = 0
            for a in range(0, G, step):
                b = min(G, a + step)
                # wait until the b-th completed group's copy is done
                sync.wait_ge(s_po, b)
                sync.dma_start(
                    out=out_d[:, a * D:b * D], in_=out_sb[:, a:b, :],
                ).then_inc(s_od, 16)
                nod += 16
            sync.wait_ge(s_od, nod)

        @blockC.tensor
        def _(tensor):
            tensor.wait_ge(s_wp, 16)
            ci = 0
            for bi, (ga, gb) in enumerate(batches):
                while ga >= chunks[ci][1]:
                    ci += 1
                if ga == chunks[ci][0]:
                    tensor.wait_ge(s_xt[ci], 16)
                if bi >= 1:
                    tensor.wait_ge(s_yA, bi)
                for j, nt in enumerate(range(ga, gb)):
                    for k in range(K):
                        mm = tensor.matmul(
                            psA[:, j, :],
                            xT_sb[:, k, nt * P:(nt + 1) * P],
                            wp_sb[:, k * D:(k + 1) * D],
                            start=(j == 0 and k == 0),
                            stop=(nt == gb - 1 and k == K - 1),
                        )
                mm.then_inc(s_mmA, 1)
            # phase C
            tensor.wait_ge(s_pz, 1)
            for r in runs:
                g, pos, nt = r["g"], r["pos"], r["nt"]
                tensor.wait_ge(s_dve, r["dve_target"])
                for i in range(nt):
                    posr = r["posr"] + i
                    mm = tensor.matmul(
                        psc(g),
                        mbuf[:, posr, :],
                        msg[:, posr, :],
                        start=False, stop=False, skip_group_check=True,
                    )
                mm.then_inc(s_mm, 1)

        @blockC.scalar
        def _(scalar):
            # phase A: per-batch psum -> bf16 y_sb copy
            for bi, (ga, gb) in enumerate(batches):
                scalar.wait_ge(s_mmA, bi + 1)
                scalar.activation(
                    out=y_sb[:, ga:gb, :], in_=psA[:, 0:gb - ga, :],
                    func=COPY,
                ).then_inc(s_yA, 1)
            # phase C: completed-group copies, in completion order
            for j, g in enumerate(comp_order):
                scalar.wait_ge(s_mm, int(run_end[last_pos[g]]))
                scalar.copy(out_sb[:, j, :], psc(g)).then_inc(s_po, 1)

        @blockC.vector
        def _(vector):
            vector.memset(psC[:], 0.0).then_inc(s_pz, 1)
            vector.memset(gbuf[:], 0.0).then_inc(s_ms, 1)
            vector.wait_ge(s_meta, 64)
            cur_call = None
            for r in runs:
                pos, nt, posr, sg2 = r["pos"], r["nt"], r["posr"], r["sigma"]
                cc = r["call"]
                if cc is not cur_call:
                    vector.wait_ge(s_g[cc["queue"]], 16 * (cc["qidx"] + 1))
                    cur_call = cc
                dstb = dst_sb[:, pos:pos + nt].unsqueeze(2).to_broadcast(
                    [P, nt, P])
                iotb = iota_sb[:, :].unsqueeze(1).to_broadcast([P, nt, P])
                vector.tensor_tensor(
                    out=mbuf[:, posr:posr + nt, :], in0=dstb, in1=iotb,
                    op=mybir.AluOpType.is_equal,
                ).then_inc(s_dve, 1)
                wgtb = wgt_sb[:, pos:pos + nt].unsqueeze(2).to_broadcast(
                    [P, nt, D])
                vector.tensor_tensor(
                    out=msg[:, posr:posr + nt, :],
                    in0=gbuf[:, posr:posr + nt, sg2 * D:(sg2 + 1) * D],
                    in1=wgtb,
                    op=mybir.AluOpType.mult,
                ).then_inc(s_dve, 1)

        @blockC.gpsimd
        def _(gpsimd):
            gpsimd.load_library(library_config.mlp)

            def ag(i):
                gpsimd.wait_ge(s_ydma[i], 32)
                gpsimd.collective_compute(
                    "AllGather",
                    mybir.AluOpType.bypass,
                    replica_groups=[list(range(C))],
                    ins=[y_own[obase[i]:obase[i + 1]].opt()],
                    outs=[y_all[abase[i]:abase[i + 1]].opt()],
                ).then_inc(s_cc)

            def do_call(ccall):
                pos, nt = ccall["pos"], ccall["nt"]
                gpsimd.wait_ge(s_cc, ccall["epoch"] + 1)
                if ccall["guard"] > 0:
                    gpsimd.wait_ge(s_mm, ccall["guard"])
                posr = ccall["posr"]
                gpsimd.dma_gather(
                    gbuf[:, posr:posr + nt, :],
                    y_rows[0:int(tbase[ccall["epoch"] + 1]), :],
                    gidx_sb[:, 8 * pos: 8 * (pos + nt)],
                    nt * P, nt * P, 2 * D,
                    single_packet=False, queue_num=ccall["queue"],
                ).then_inc(s_g[ccall["queue"]], 16)

            by_ep = [[cl for cl in calls if cl["epoch"] == ep]
                     for ep in range(NCHUNK)]
            gpsimd.wait_ge(s_ms, 1)
            ag(0)
            if NCHUNK > 1:
                ag(1)
            for ep in range(NCHUNK):
                for ccall in by_ep[ep]:
                    do_call(ccall)
                if ep + 2 < NCHUNK:
                    ag(ep + 2)

    nc.compile()
    return nc


def _assemble(cfg, plan, outs):
    D, G, NS = cfg.D, cfg.G, cfg.NS
    comp_order = plan["comp_order"]
    full = np.empty((cfg.N, D), np.float32)
    for c in range(cfg.C):
        o = outs[c]["out"].reshape(P, G, D)
        per_g = np.empty((G, P, D), np.float32)
        for j, g in enumerate(comp_order):
            per_g[g] = o[:, j, :]
        flat = per_g.transpose(0, 1, 2).reshape(cfg.NS_PAD, D)
        full[c * NS:(c + 1) * NS] = flat[:NS]
    return full


def gnn_kernel(x, edge_src, edge_dst, edge_weight, w_bases, w_rel,
               cfg=None, trace=False):
    if cfg is None:
        cfg = Cfg(N=50000, E=800000)
    plan, in_maps = plan_and_pack(cfg, np.asarray(x), np.asarray(edge_src),
                                  np.asarray(edge_dst), np.asarray(edge_weight),
                                  np.asarray(w_bases), np.asarray(w_rel))
    nc = build_nc(cfg, plan)
    res = run_bass_kernel_spmd(nc, in_maps, list(range(cfg.C)), trace=trace)
    return _assemble(cfg, plan, res.results), res


def kernel(x, edge_src, edge_dst, edge_weight, w_bases, w_rel):
    """Full inputs in, full output out. Shards across 8 NeuronCores inside."""
    full, _ = gnn_kernel(x, edge_src, edge_dst, edge_weight, w_bases, w_rel)
    return full
